# revision 5
# baseline (speedup 1.0000x reference)
"""Trainium2 Bass kernel — Kuramoto step, col-tiled fp8 PE version.

delta[b] = ((C[b] @ sin(ph_b)) * cos(ph_b) - (C[b] @ cos(ph_b)) * sin(ph_b)) / N

Col-tiling (tile_position=(0, 32g)) runs 4 batches' matvecs CONCURRENTLY on
the 128x32-tiled PE array: aggregate ingest 4x128 fp8/cycle — matches warm
DoubleRow even at the cold (1.2 GHz) HAM clock, ~2x at warm.  This makes the
kernel insensitive to the PE clock state (the main source of run variance).

Staged layouts (host-built):
  coupT_s [8c, 128p, (4g, 2ktl, 1024i)] fp8: chunk c=(q,k2) holds j-rows
      (2*k2+ktl)*128+p of batches 4q+g — 8 KiB contiguous per partition.
  sc_s [128, (8b, 8kt, 2m)] fp8: m=0 sin(ph[b, kt*128+p]), m=1 cos.
  trig_s [128, (2q, 2h, 512i)] f32: rows 32g+0 = cos(ph[4q+g, h*512+i]),
      rows 32g+1 = -sin(...), other rows 0.
  sel_s [128, 4] bf16: sel[32g+r, g] = 1 for r in {0,1} (pair-sum weights).
Output delta_s [8192] f32 (omega added on host).
"""
import numpy as np
import ml_dtypes

import concourse.bass as bass
import concourse.bacc as bacc
import concourse.mybir as mybir
import concourse.tile as tile
from concourse import bass_utils

B, N = 64, 1024
NCORES = 8
BPC = B // NCORES          # 8 batches per core
KT = 8                     # j tiles of 128 per batch
KT2 = 4
P = 128

f32 = mybir.dt.float32
bf16 = mybir.dt.bfloat16
A = mybir.AluOpType

CDT = mybir.dt.float8e4
CDT_NP = ml_dtypes.float8_e4m3

_cached = None


def _build():
    nc = bacc.Bacc("TRN2", target_bir_lowering=False)

    ct_d = nc.dram_tensor("coupT_s", (8, P, 4 * 2 * N), CDT,
                          kind="ExternalInput")
    sc_d = nc.dram_tensor("sc_s", (P, BPC * KT * 2), CDT,
                          kind="ExternalInput")
    trig_d = nc.dram_tensor("trig_s", (P, 4 * 512), f32, kind="ExternalInput")
    sel_d = nc.dram_tensor("sel_s", (P, 4), bf16, kind="ExternalInput")
    out_d = nc.dram_tensor("delta_s", (BPC * N,), f32, kind="ExternalOutput")

    out_v = out_d[:].rearrange("(b j) -> b j", b=BPC)                # [8,1024]

    with tile.TileContext(nc) as tc:
        with (
            tc.tile_pool(name="small", bufs=1) as small,
            tc.tile_pool(name="cbuf", bufs=1) as cbuf,
            tc.tile_pool(name="ppool", bufs=1, space="PSUM") as ppool,
            tc.tile_pool(name="psel", bufs=2, space="PSUM") as pselp,
            tc.tile_pool(name="fpool", bufs=4) as fpool,
        ):
            # ---------- C^T chunk streams (Sync ring, depth-2 chain) --------
            cts = []
            ct_dmas = []
            H = 2 * 2 * N
            for c in range(8):
                ct_c = cbuf.tile([P, 4 * 2 * N], CDT, tag=f"ct{c}")
                if c == 7:
                    # split the last chunk (batches g0-1 land first) so the
                    # matmul trail after the final byte is halved
                    ct_dmas.append(nc.sync.dma_start(
                        out=ct_c[:, 0:H], in_=ct_d[7][:, 0:H]))
                    ct_dmas.append(nc.sync.dma_start(
                        out=ct_c[:, H:], in_=ct_d[7][:, H:]))
                else:
                    ct_dmas.append(nc.sync.dma_start(out=ct_c, in_=ct_d[c]))
                cts.append(ct_c.rearrange("p (g ktl i) -> p g ktl i", g=4,
                                          ktl=2))
            for k in range(2, len(ct_dmas)):
                tile.add_dep_helper(ct_dmas[k].ins, ct_dmas[k - 2].ins,
                                    reason="serialize C^T stream")

            # ---------- small loads on the GpSimd SWDGE queue ---------------
            sc_all = small.tile([P, BPC * KT * 2], CDT)
            nc.gpsimd.dma_start(out=sc_all, in_=sc_d[:])
            sc_v = sc_all.rearrange("p (b kt m) -> p b kt m", b=BPC, kt=KT)
            trig = small.tile([P, 4 * 512], f32)
            nc.gpsimd.dma_start(out=trig, in_=trig_d[:])
            sel = small.tile([P, 4], bf16)
            nc.gpsimd.dma_start(out=sel, in_=sel_d[:])

            # psum accumulators [128, 512] (1 bank each), zeroed so the
            # unwritten partition rows read as 0.0 (not stale NaN bits)
            pq = {}
            for q in range(2):
                for h in range(2):
                    t = ppool.tile([P, 512], f32, tag=f"pq{q}{h}")
                    nc.vector.memset(t, 0.0)
                    pq[(q, h)] = t

            # ---------- PE warmup (plain MMs trip the HAM clock) ------------
            wsrc = small.tile([P, 512], bf16)
            nc.vector.memset(wsrc, 0.0)
            wpt = pselp.tile([4, 512], f32, tag="psel")
            for _ in range(15):
                nc.tensor.matmul(wpt, wsrc[:, 0:4], wsrc,
                                 start=True, stop=True)

            # ---------- main loop: 4-way col-tiled matvecs ------------------
            for q in range(2):
                for k2 in range(KT2):
                    c = 4 * q + k2
                    for ktl in range(2):
                        kt = 2 * k2 + ktl
                        for h in range(2):
                            for g in range(4):
                                nc.tensor.matmul(
                                    pq[(q, h)][32 * g:32 * g + 2, :],
                                    sc_v[:, 4 * q + g, kt, :],
                                    cts[c][:, g, ktl,
                                           h * 512:(h + 1) * 512],
                                    start=(k2 == 0 and ktl == 0),
                                    stop=(k2 == KT2 - 1 and ktl == 1),
                                    tile_position=(0, 32 * g))

                # finalize per (q, h): full-width ops over 4 batches at once
                for h in range(2):
                    prod = fpool.tile([P, 512], bf16, tag="prod")
                    nc.vector.scalar_tensor_tensor(
                        out=prod, in0=pq[(q, h)], scalar=1.0 / N,
                        in1=trig[:, (2 * q + h) * 512:(2 * q + h + 1) * 512],
                        op0=A.mult, op1=A.mult)
                    ps = pselp.tile([4, 512], f32, tag="psel")
                    nc.tensor.matmul(ps, sel, prod, start=True, stop=True)
                    dq = fpool.tile([4, 512], f32, tag="dq")
                    nc.vector.tensor_copy(out=dq, in_=ps)
                    nc.scalar.dma_start(
                        out=out_v[4 * q:4 * q + 4, h * 512:(h + 1) * 512],
                        in_=dq)

    nc.compile()
    return nc


def _host_prep(couplings):
    """[8, N, N] f32 -> staged chunks [8, 128, 4*2*1024] fp8.

    chunk c = (q, k2): element [c, p, g, ktl, i] = C[4q+g][i, (2k2+ktl)*128+p]
    """
    cf8 = couplings.astype(CDT_NP)                       # [b, i, j]
    ct = cf8.reshape(2, 4, N, KT2, 2, P)                 # [q, g, i, k2, ktl, p]
    ct = np.ascontiguousarray(ct.transpose(0, 3, 5, 1, 4, 2))
    return ct.reshape(8, P, 4 * 2 * N)


def _host_trig(ph):
    """ph [8, N] -> (sc_s, trig_s, sel_s)."""
    s = np.sin(ph)
    c = np.cos(ph)
    sc = np.zeros((P, BPC, KT, 2), dtype=np.float32)
    sc[:, :, :, 0] = s.reshape(BPC, KT, P).transpose(2, 0, 1)
    sc[:, :, :, 1] = c.reshape(BPC, KT, P).transpose(2, 0, 1)
    sc8 = sc.reshape(P, BPC * KT * 2).astype(CDT_NP)

    trig = np.zeros((P, 2, 2, 512), dtype=np.float32)
    for g in range(4):
        for q in range(2):
            b = 4 * q + g
            trig[32 * g + 0, q, :, :] = c[b].reshape(2, 512)
            trig[32 * g + 1, q, :, :] = -s[b].reshape(2, 512)
    trig = trig.reshape(P, 4 * 512)

    sel = np.zeros((P, 4), dtype=np.float32)
    for g in range(4):
        sel[32 * g + 0, g] = 1.0
        sel[32 * g + 1, g] = 1.0
    sel = sel.astype(ml_dtypes.bfloat16)
    return sc8, trig, sel


def make_in_maps(ph, cp, om):
    in_maps = []
    for k in range(NCORES):
        sl = slice(k * BPC, (k + 1) * BPC)
        sc8, trig, sel = _host_trig(ph[sl])
        in_maps.append({
            "coupT_s": _host_prep(cp[sl]),
            "sc_s": sc8,
            "trig_s": trig,
            "sel_s": sel,
        })
    return in_maps


def kernel(t=None, phase=None, couplings=None, omega=None, **kw):
    global _cached
    if _cached is None:
        _cached = _build()
    nc = _cached

    phase = np.asarray(phase, dtype=np.float32)
    couplings = np.asarray(couplings, dtype=np.float32)
    omega = np.asarray(omega, dtype=np.float32).ravel()

    ph = phase.reshape(B, N)
    in_maps = make_in_maps(ph, couplings, None)
    res = bass_utils.run_bass_kernel_spmd(nc, in_maps,
                                          core_ids=list(range(NCORES)))
    out = np.concatenate([r["delta_s"] for r in res.results])
    return out.astype(np.float32) + omega
